# revision 10
# baseline (speedup 1.0000x reference)
"""Trainium2 Bass kernel for nn_MultiHeadAttention (B=4, S=2048, D=1024, H=16).

Sharding: 8 cores, core c handles batch b=c//2 and query-row half qh=c%2
(1024 query rows), with all 16 heads and the full 2048-key context for that
batch.  No collectives: each core produces a disjoint [1024, 1024] output slab.

v2 design (vs v1): everything SBUF-resident in bf16 (no DRAM spill
round-trips), weights/activations host-cast to bf16 (halves input DMA, makes
PE transposes 1 cycle/row, keeps matmuls at full rate), ACT engine runs ONLY
the softmax exp (the hard floor: 33.5M elems/core at 1 elem/cycle/lane
@1.2GHz), and phase A (projections) is software-pipelined with phase B
(attention) at key-token-block granularity so the exp stream starts ~30us in
instead of after all projections.

Per-core dataflow:
  - X.T tiles produced by PE transpose-mode (bf16, 1 cyc/row), drained
    PSUM->SBUF by DVE in groups of 4.
  - Q/K projections: out = W[it,ot].T @ X.T -> feature-major Q.T/K.T (bf16,
    SBUF-resident), bias added during the DVE eviction.
  - V projection: out = Xblk @ W -> token-major V with a ones column per head
    (65-wide stride) for the softmax denominator; bias via DVE eviction.
  - Scores: per head-pair, two K=64 matmuls packed into PE row-halves via
    tile_position (concurrent on HW), exp on ACT with the 1/8 scale folded
    in, bf16 P out.
  - AV: x_aug = V_aug.T @ P accumulated over 16 key tiles; row 64 is the
    denominator. Normalize: DVE reciprocal + GPSIMD partition-broadcast +
    DVE multiply -> bf16 x.T.
  - Output projection: out = x.T.T @ Wo + bo accumulated over 8 head-pair
    blocks, fp32 result DMA'd out.
"""

import os
import sys

import numpy as np

sys.path.insert(0, "/opt/trn_rl_repo")

import concourse.bass as bass  # noqa: E402
import concourse.tile as tile  # noqa: E402
from concourse import bacc, mybir  # noqa: E402
from concourse.bass_utils import run_bass_kernel_spmd  # noqa: E402
from concourse.masks import make_identity  # noqa: E402

B, S, D, H = 4, 2048, 1024, 16
HD = D // H          # 64
P = 128
SQ = S // 2          # query rows per core
SK = S               # key rows per core
NIT = D // P         # 8 input-feature tiles
NOT = D // P         # 8 output-feature tiles (= head pairs)
KT = SK // P         # 16 key-token tiles
NP = H // 2          # 8 head pairs
VW = HD + 1          # 65: head slice of V plus ones column
TB = 512             # token block for projections
NKB = SK // TB       # 4 key token blocks
NQB2 = SQ // TB      # 2 query token blocks

F32 = mybir.dt.float32
BF16 = mybir.dt.bfloat16
EXP = mybir.ActivationFunctionType.Exp
ADD = mybir.AluOpType.add
MULT = mybir.AluOpType.mult

_CACHE: dict = {}


def _emit(tc, io, tag=""):
    nc = tc.nc

    with (
        tc.tile_pool(name="persist" + tag, bufs=1) as persist,
        tc.tile_pool(name="consts" + tag, bufs=1) as consts,
        tc.tile_pool(name="wbuf" + tag, bufs=3) as wpool,
        tc.tile_pool(name="xrow" + tag, bufs=2) as xrow_pool,
        tc.tile_pool(name="xtblk" + tag, bufs=2) as xt_pool,
        tc.tile_pool(name="pexp" + tag, bufs=3) as pe_pool,
        tc.tile_pool(name="rcp" + tag, bufs=2) as rcp_pool,
        tc.tile_pool(name="rbc" + tag, bufs=2) as rb_pool,
        tc.tile_pool(name="cstage" + tag, bufs=2) as cst_pool,
        tc.tile_pool(name="ps_a" + tag, bufs=2, space="PSUM") as ps_a,
        tc.tile_pool(name="s_ps" + tag, bufs=2, space="PSUM") as s_psum,
        tc.tile_pool(name="x_ps" + tag, bufs=2, space="PSUM") as x_psum,
    ):
        # ---- persistent SBUF tensors ----
        kt_sb = persist.tile([P, NP, SK], BF16, tag="ktr")   # K.T [dim, pair, key]
        qt_sb = persist.tile([P, NP, SQ], BF16, tag="qtr")   # Q.T [dim, pair, query]
        v_sb = persist.tile([P, KT, NP * 2 * VW], BF16, tag="vr")  # V+ones tokmajor
        xtn_sb = persist.tile([P, NP, SQ], BF16, tag="xtn")  # normalized x.T
        wo_sb = persist.tile([P, NP, D], BF16, tag="wo")

        # ---- constants ----
        ident_f32 = consts.tile([P, P], F32, tag="identf")
        make_identity(nc, ident_f32)
        ident = consts.tile([P, P], BF16, tag="ident")
        nc.vector.tensor_copy(ident[:], ident_f32[:])
        bqt = consts.tile([P, NOT], F32, tag="bqt")
        nc.sync.dma_start(out=bqt[:], in_=io["bq"].rearrange("(a p) -> p a", p=P))
        bkt = consts.tile([P, NOT], F32, tag="bkt")
        nc.sync.dma_start(out=bkt[:], in_=io["bk"].rearrange("(a p) -> p a", p=P))
        bv_row = consts.tile([1, D], BF16, tag="bvr")
        nc.sync.dma_start(out=bv_row[:], in_=io["bv"].rearrange("(a d) -> a d", a=1))
        bo_row = consts.tile([1, D], BF16, tag="bor")
        nc.sync.dma_start(out=bo_row[:], in_=io["bo"].rearrange("(a d) -> a d", a=1))
        bv_bcast = consts.tile([P, D], BF16, tag="bvb")
        nc.gpsimd.partition_broadcast(bv_bcast[:], bv_row[0:1, :])
        bo_bcast = consts.tile([P, D], BF16, tag="bob")
        nc.gpsimd.partition_broadcast(bo_bcast[:], bo_row[0:1, :])

        # ones columns of V_aug, set once (disjoint from the V eviction region)
        ones_view = v_sb[:].rearrange("p k (g c) -> p k g c", c=VW)
        nc.vector.memset(ones_view[:, :, :, HD : HD + 1], 1.0)

        def load_w(which):
            w_sb = wpool.tile([P, NIT, D], BF16, tag="w", name=f"w_{which}")
            for it in range(NIT):
                nc.sync.dma_start(
                    out=w_sb[:, it], in_=io[which][it * P : (it + 1) * P, :]
                )
            return w_sb

        def transpose_block(x_ap, t0, dst):
            """dst[:, it, ts*128:...] = x_ap[t0:t0+512, :].T via PE transposes.

            PSUM staging in [P, 512] group tiles (4 transposes per drain) so
            the DVE drain rate keeps pace with PE at minimal PSUM footprint.
            """
            for ts in range(4):
                xrow = xrow_pool.tile([P, D], BF16, tag="xrow", name=f"xr_{t0}_{ts}")
                nc.sync.dma_start(
                    out=xrow[:], in_=x_ap[t0 + ts * P : t0 + (ts + 1) * P, :]
                )
                for a in range(2):
                    tp = ps_a.tile([P, 4, P], BF16, tag="pj", name=f"tp_{t0}_{ts}_{a}")
                    for j in range(4):
                        it = 4 * a + j
                        nc.tensor.transpose(
                            tp[:, j], xrow[:, it * P : (it + 1) * P], ident[:]
                        )
                    nc.vector.tensor_copy(
                        dst[:, 4 * a : 4 * a + 4, ts * P : (ts + 1) * P], tp[:]
                    )

        def qk_proj_block(w_sb, xt_blk, dst_sb, bias, tb):
            """dst_sb[:, ot, tb*512:...] = (W.T @ Xblk.T) + bias, bf16 out."""
            for ot in range(NOT):
                ps = ps_a.tile([P, TB], F32, tag="pj", name=f"p_{tb}_{ot}")
                for it in range(NIT):
                    nc.tensor.matmul(
                        ps[:],
                        w_sb[:, it, ot * P : (ot + 1) * P],
                        xt_blk[:, it],
                        start=(it == 0),
                        stop=(it == NIT - 1),
                    )
                nc.vector.tensor_scalar_add(
                    dst_sb[:, ot, tb * TB : (tb + 1) * TB],
                    ps[:],
                    bias[:, ot : ot + 1],
                )

        def v_proj_block(w_sb, xt_blk, tb):
            """v_sb[:, kt, ...] = (Xblk @ Wv) + bv for token tiles of block tb."""
            for ts in range(4):
                kt = tb * 4 + ts
                for ob in range(2):
                    ps = ps_a.tile([P, TB], F32, tag="pj", name=f"vp_{kt}_{ob}")
                    for it in range(NIT):
                        nc.tensor.matmul(
                            ps[:],
                            xt_blk[:, it, ts * P : (ts + 1) * P],
                            w_sb[:, it, ob * TB : (ob + 1) * TB],
                            start=(it == 0),
                            stop=(it == NIT - 1),
                        )
                    dst = v_sb[:, kt, ob * 4 * 2 * VW : (ob + 1) * 4 * 2 * VW]
                    nc.vector.tensor_tensor(
                        dst.rearrange("p (g c) -> p g c", c=VW)[:, :, 0:HD],
                        ps[:].rearrange("p (g c) -> p g c", c=HD),
                        bv_bcast[:, ob * TB : (ob + 1) * TB].rearrange(
                            "p (g c) -> p g c", c=HD
                        ),
                        op=ADD,
                    )

        attn_state: dict = {}

        def emit_av(qb, pr, xs, pending):
            """AV matmuls for a (pexp, kt) unit produced by a prior exp."""
            pexp, kt = pending
            for h2 in range(2):
                nc.tensor.matmul(
                    xs[h2][:],
                    v_sb[:, kt, (2 * pr + h2) * VW : (2 * pr + h2 + 1) * VW],
                    pexp[:, h2 * TB : (h2 + 1) * TB],
                    start=(kt == 0),
                    stop=(kt == KT - 1),
                )

        def attn_chunk(qb, pr, kts):
            """Scores+exp+AV for key tiles `kts` of block (qb, pr).

            Each kt's AV is emitted during the NEXT kt's scores so the PE
            never FIFO-blocks on the exp.
            """
            if kts[0] == 0:
                attn_state[(qb, pr)] = (
                    [
                        x_psum.tile([VW, TB], F32, tag="xa", name=f"x_{qb}_{pr}_{h2}")
                        for h2 in range(2)
                    ],
                    [None],
                )
            xs, pend = attn_state[(qb, pr)]
            for kt in kts:
                sp = s_psum.tile([P, 1024], F32, tag="sp", name=f"sp_{qb}_{pr}_{kt}")
                for h2 in range(2):
                    nc.tensor.matmul(
                        sp[:, h2 * TB : (h2 + 1) * TB],
                        kt_sb[h2 * HD : (h2 + 1) * HD, pr, kt * P : (kt + 1) * P],
                        qt_sb[h2 * HD : (h2 + 1) * HD, pr, qb * TB : (qb + 1) * TB],
                        tile_position=(h2 * HD, 0),
                    )
                if pend[0] is not None:
                    emit_av(qb, pr, xs, pend[0])
                pexp = pe_pool.tile([P, 1024], BF16, tag="pe", name=f"pe_{qb}_{pr}_{kt}")
                nc.scalar.activation(pexp[:], sp[:], EXP, scale=1.0 / 8.0)
                pend[0] = (pexp, kt)
            if kts[-1] == KT - 1:
                emit_av(qb, pr, xs, pend[0])
                pend[0] = None
                xs = attn_state[(qb, pr)][0]
                for h2 in range(2):
                    rcp = rcp_pool.tile([1, TB], BF16, tag="rc", name=f"rc_{qb}_{pr}_{h2}")
                    with nc.allow_low_precision(
                        reason="bf16 softmax denominator reciprocal (0.4% el)"
                    ):
                        nc.vector.reciprocal(rcp[:], xs[h2][HD : HD + 1, :])
                    rb = rb_pool.tile([HD, TB], BF16, tag="rb", name=f"rb_{qb}_{pr}_{h2}")
                    nc.gpsimd.partition_broadcast(rb[:], rcp[0:1, :])
                    nc.vector.tensor_tensor(
                        xtn_sb[h2 * HD : (h2 + 1) * HD, pr, qb * TB : (qb + 1) * TB],
                        xs[h2][0:HD, :],
                        rb[:],
                        op=MULT,
                    )
                del attn_state[(qb, pr)]

        def c_chunk(qt, ob):
            """One [128, 512] tile of out = x.T.T @ Wo + bo."""
            ps = ps_a.tile([P, TB], F32, tag="pj", name=f"op_{qt}_{ob}")
            for pr in range(NP):
                nc.tensor.matmul(
                    ps[:],
                    xtn_sb[:, pr, qt * P : (qt + 1) * P],
                    wo_sb[:, pr, ob * TB : (ob + 1) * TB],
                    start=(pr == 0),
                    stop=(pr == NP - 1),
                )
            st = cst_pool.tile([P, TB], F32, tag="os", name=f"os_{qt}_{ob}")
            nc.vector.tensor_tensor(
                st[:], ps[:], bo_bcast[:, ob * TB : (ob + 1) * TB], op=ADD
            )
            nc.sync.dma_start(
                out=io["out"][qt * P : (qt + 1) * P, ob * TB : (ob + 1) * TB],
                in_=st[:],
            )

        # ---------------- pipelined emission ----------------
        # Head: Q(tb0) so scores' moving operand exists, then K(tb0)/V(tb0)
        # unblock the first attention chunk; remaining K/V blocks interleave
        # with the attention kt-chunks they feed.
        wq = load_w("wq")
        xtq0 = xt_pool.tile([P, NIT, TB], BF16, tag="xt", name="xtq_0")
        transpose_block(io["xq"], 0, xtq0)
        qk_proj_block(wq, xtq0, qt_sb, bqt, 0)

        wk = load_w("wk")
        xtk = xt_pool.tile([P, NIT, TB], BF16, tag="xt", name="xtk_0")
        transpose_block(io["xk"], 0, xtk)
        qk_proj_block(wk, xtk, kt_sb, bkt, 0)

        wv = load_w("wv")
        xtv = xt_pool.tile([P, NIT, TB], BF16, tag="xt", name="xtv_0")
        transpose_block(io["xv"], 0, xtv)
        v_proj_block(wv, xtv, 0)

        attn_chunk(0, 0, [0, 1, 2, 3])
        for tb in range(1, NKB):
            xtk = xt_pool.tile([P, NIT, TB], BF16, tag="xt", name=f"xtk_{tb}")
            transpose_block(io["xk"], tb * TB, xtk)
            qk_proj_block(wk, xtk, kt_sb, bkt, tb)
            xtv = xt_pool.tile([P, NIT, TB], BF16, tag="xt", name=f"xtv_{tb}")
            transpose_block(io["xv"], tb * TB, xtv)
            v_proj_block(wv, xtv, tb)
            attn_chunk(0, 0, [4 * tb, 4 * tb + 1, 4 * tb + 2, 4 * tb + 3])

        # remaining attention, with the rest of phase A and the qb0 output
        # projection spread through the B blocks as PE filler
        attn_chunk(0, 1, list(range(KT)))
        xtq1 = xt_pool.tile([P, NIT, TB], BF16, tag="xt", name="xtq_1")
        transpose_block(io["xq"], TB, xtq1)
        attn_chunk(0, 2, list(range(KT)))
        qk_proj_block(wq, xtq1, qt_sb, bqt, 1)
        attn_chunk(0, 3, list(range(KT)))
        for it in range(NP):
            nc.sync.dma_start(
                out=wo_sb[:, it], in_=io["wo"][it * P : (it + 1) * P, :]
            )
        for pr in range(4, NP):
            attn_chunk(0, pr, list(range(KT)))
        for pr in range(NP):
            attn_chunk(1, pr, list(range(KT)))
            if pr == 1:
                c_chunk(0, 0)
                c_chunk(0, 1)
            elif pr >= 2 and pr <= 4:
                c_chunk(pr - 1, 0)
                c_chunk(pr - 1, 1)
        for qt in range(4, 8):
            c_chunk(qt, 0)
            c_chunk(qt, 1)


def build_module(reps=1):
    key = ("nc", reps)
    if key in _CACHE:
        return _CACHE[key]
    nc = bacc.Bacc("TRN2", target_bir_lowering=False, debug=False, num_devices=8)
    io = {}
    io["xq"] = nc.dram_tensor("xq", [SQ, D], BF16, kind="ExternalInput").ap()
    io["xk"] = nc.dram_tensor("xk", [SK, D], BF16, kind="ExternalInput").ap()
    io["xv"] = nc.dram_tensor("xv", [SK, D], BF16, kind="ExternalInput").ap()
    for w in ("wq", "wk", "wv", "wo"):
        io[w] = nc.dram_tensor(w, [D, D], BF16, kind="ExternalInput").ap()
    for b in ("bq", "bk"):
        io[b] = nc.dram_tensor(b, [D], F32, kind="ExternalInput").ap()
    for b in ("bv", "bo"):
        io[b] = nc.dram_tensor(b, [D], BF16, kind="ExternalInput").ap()
    io["out"] = nc.dram_tensor("out", [SQ, D], F32, kind="ExternalOutput").ap()

    with tile.TileContext(nc) as tc:
        for rep in range(reps):
            _emit(tc, io, tag=f"_r{rep}" if rep else "")
    nc.compile()
    _CACHE[key] = nc
    return nc


def _bf16(x):
    import ml_dtypes

    return np.ascontiguousarray(np.asarray(x, np.float32).astype(ml_dtypes.bfloat16))


def make_in_maps(query, key, value, Wq, bq, Wk, bk, Wv, bv, Wo, bo):
    shared = {
        "wq": _bf16(Wq),
        "wk": _bf16(Wk),
        "wv": _bf16(Wv),
        "wo": _bf16(Wo),
        "bq": np.ascontiguousarray(np.asarray(bq, np.float32)),
        "bk": np.ascontiguousarray(np.asarray(bk, np.float32)),
        "bv": _bf16(bv),
        "bo": _bf16(bo),
    }
    qb, kb, vb = _bf16(query), _bf16(key), _bf16(value)
    in_maps = []
    for c in range(8):
        b, qh = divmod(c, 2)
        in_maps.append(
            {
                "xq": np.ascontiguousarray(qb[b, qh * SQ : (qh + 1) * SQ]),
                "xk": kb[b],
                "xv": vb[b],
                **shared,
            }
        )
    return in_maps


LAST_RESULTS = None


def kernel(query, key, value, Wq, bq, Wk, bk, Wv, bv, Wo, bo):
    global LAST_RESULTS
    nc = build_module()
    in_maps = make_in_maps(query, key, value, Wq, bq, Wk, bk, Wv, bv, Wo, bo)
    try:
        res = run_bass_kernel_spmd(nc, in_maps, core_ids=list(range(8)))
    except ModuleNotFoundError:
        # BASS_TRACE was requested but this container lacks the axon NTFF
        # profiling hook module; rerun with tracing disabled.
        os.environ["BASS_NEVER_TRACE"] = "1"
        res = run_bass_kernel_spmd(nc, in_maps, core_ids=list(range(8)))
    LAST_RESULTS = res
    out = np.empty((B, S, D), np.float32)
    for c in range(8):
        b, qh = divmod(c, 2)
        out[b, qh * SQ : (qh + 1) * SQ] = res.results[c]["out"]
    return out


# revision 14
# speedup vs baseline: 16.0662x; 16.0662x over previous
"""Trainium2 Bass kernel for nn_MultiHeadAttention (B=4, S=2048, D=1024, H=16).

Sharding: 8 cores, core c handles batch b=c//2 and query-row half qh=c%2
(1024 query rows), with all 16 heads and the full 2048-key context for that
batch.  No collectives: each core produces a disjoint [1024, 1024] output slab.

v2 design (vs v1): everything SBUF-resident in bf16 (no DRAM spill
round-trips), weights/activations host-cast to bf16 (halves input DMA, makes
PE transposes 1 cycle/row, keeps matmuls at full rate), ACT engine runs ONLY
the softmax exp (the hard floor: 33.5M elems/core at 1 elem/cycle/lane
@1.2GHz), and phase A (projections) is software-pipelined with phase B
(attention) at key-token-block granularity so the exp stream starts ~30us in
instead of after all projections.

Per-core dataflow:
  - X.T tiles produced by PE transpose-mode (bf16, 1 cyc/row), drained
    PSUM->SBUF by DVE in groups of 4.
  - Q/K projections: out = W[it,ot].T @ X.T -> feature-major Q.T/K.T (bf16,
    SBUF-resident), bias added during the DVE eviction.
  - V projection: out = Xblk @ W -> token-major V with a ones column per head
    (65-wide stride) for the softmax denominator; bias via DVE eviction.
  - Scores: per head-pair, two K=64 matmuls packed into PE row-halves via
    tile_position (concurrent on HW), exp on ACT with the 1/8 scale folded
    in, bf16 P out.
  - AV: x_aug = V_aug.T @ P accumulated over 16 key tiles; row 64 is the
    denominator. Normalize: DVE reciprocal + GPSIMD partition-broadcast +
    DVE multiply -> bf16 x.T.
  - Output projection: out = x.T.T @ Wo + bo accumulated over 8 head-pair
    blocks, fp32 result DMA'd out.
"""

import os
import sys

import numpy as np

sys.path.insert(0, "/opt/trn_rl_repo")

import concourse.bass as bass  # noqa: E402
import concourse.tile as tile  # noqa: E402
from concourse import bacc, mybir  # noqa: E402
from concourse.bass_utils import run_bass_kernel_spmd  # noqa: E402
from concourse.masks import make_identity  # noqa: E402

B, S, D, H = 4, 2048, 1024, 16
HD = D // H          # 64
P = 128
SQ = S // 2          # query rows per core
SK = S               # key rows per core
NIT = D // P         # 8 input-feature tiles
NOT = D // P         # 8 output-feature tiles (= head pairs)
KT = SK // P         # 16 key-token tiles
NP = H // 2          # 8 head pairs
VW = HD + 1          # 65: head slice of V plus ones column
TB = 512             # token block for projections
NKB = SK // TB       # 4 key token blocks
NQB2 = SQ // TB      # 2 query token blocks

F32 = mybir.dt.float32
BF16 = mybir.dt.bfloat16
EXP = mybir.ActivationFunctionType.Exp
ADD = mybir.AluOpType.add
MULT = mybir.AluOpType.mult

_CACHE: dict = {}


def _emit(tc, io, tag=""):
    nc = tc.nc

    with (
        tc.tile_pool(name="persist" + tag, bufs=1) as persist,
        tc.tile_pool(name="consts" + tag, bufs=1) as consts,
        tc.tile_pool(name="wbuf" + tag, bufs=3) as wpool,
        tc.tile_pool(name="xrow" + tag, bufs=2) as xrow_pool,
        tc.tile_pool(name="xtblk" + tag, bufs=2) as xt_pool,
        tc.tile_pool(name="pexp" + tag, bufs=3) as pe_pool,
        tc.tile_pool(name="rcp" + tag, bufs=2) as rcp_pool,
        tc.tile_pool(name="rbc" + tag, bufs=2) as rb_pool,
        tc.tile_pool(name="cstage" + tag, bufs=2) as cst_pool,
        tc.tile_pool(name="ps_a" + tag, bufs=2, space="PSUM") as ps_a,
        tc.tile_pool(name="s_ps" + tag, bufs=2, space="PSUM") as s_psum,
        tc.tile_pool(name="x_ps" + tag, bufs=2, space="PSUM") as x_psum,
    ):
        # ---- persistent SBUF tensors ----
        kt_sb = persist.tile([P, NP, SK], BF16, tag="ktr")   # K.T [dim, pair, key]
        qt_sb = persist.tile([P, NP, SQ], BF16, tag="qtr")   # Q.T [dim, pair, query]
        v_sb = persist.tile([P, KT, NP * 2 * VW], BF16, tag="vr")  # V+ones tokmajor
        xtn_sb = persist.tile([P, NP, SQ], BF16, tag="xtn")  # normalized x.T
        wo_sb = persist.tile([P, NP, D], BF16, tag="wo")

        # ---- constants ----
        ident_f32 = consts.tile([P, P], F32, tag="identf")
        make_identity(nc, ident_f32)
        ident = consts.tile([P, P], BF16, tag="ident")
        nc.vector.tensor_copy(ident[:], ident_f32[:])
        bqt = consts.tile([P, NOT], F32, tag="bqt")
        nc.sync.dma_start(out=bqt[:], in_=io["bq"].rearrange("(a p) -> p a", p=P))
        bkt = consts.tile([P, NOT], F32, tag="bkt")
        nc.sync.dma_start(out=bkt[:], in_=io["bk"].rearrange("(a p) -> p a", p=P))
        bv_row = consts.tile([1, D], BF16, tag="bvr")
        nc.sync.dma_start(out=bv_row[:], in_=io["bv"].rearrange("(a d) -> a d", a=1))
        bo_row = consts.tile([1, D], BF16, tag="bor")
        nc.sync.dma_start(out=bo_row[:], in_=io["bo"].rearrange("(a d) -> a d", a=1))
        bv_bcast = consts.tile([P, D], BF16, tag="bvb")
        nc.gpsimd.partition_broadcast(bv_bcast[:], bv_row[0:1, :])
        bo_bcast = consts.tile([P, D], BF16, tag="bob")
        nc.gpsimd.partition_broadcast(bo_bcast[:], bo_row[0:1, :])

        # ones columns of V_aug, set once (disjoint from the V eviction region)
        ones_view = v_sb[:].rearrange("p k (g c) -> p k g c", c=VW)
        nc.vector.memset(ones_view[:, :, :, HD : HD + 1], 1.0)

        def load_w(which):
            w_sb = wpool.tile([P, NIT, D], BF16, tag="w", name=f"w_{which}")
            for it in range(NIT):
                nc.sync.dma_start(
                    out=w_sb[:, it], in_=io[which][it * P : (it + 1) * P, :]
                )
            return w_sb

        def transpose_block(x_ap, t0, dst):
            """dst[:, it, ts*128:...] = x_ap[t0:t0+512, :].T via PE transposes.

            PSUM staging in [P, 512] group tiles (4 transposes per drain) so
            the DVE drain rate keeps pace with PE at minimal PSUM footprint.
            """
            for ts in range(4):
                xrow = xrow_pool.tile([P, D], BF16, tag="xrow", name=f"xr_{t0}_{ts}")
                nc.sync.dma_start(
                    out=xrow[:], in_=x_ap[t0 + ts * P : t0 + (ts + 1) * P, :]
                )
                for a in range(2):
                    tp = ps_a.tile([P, 4, P], BF16, tag="pj", name=f"tp_{t0}_{ts}_{a}")
                    for j in range(4):
                        it = 4 * a + j
                        nc.tensor.transpose(
                            tp[:, j], xrow[:, it * P : (it + 1) * P], ident[:]
                        )
                    nc.vector.tensor_copy(
                        dst[:, 4 * a : 4 * a + 4, ts * P : (ts + 1) * P], tp[:]
                    )

        def qk_proj_block(w_sb, xt_blk, dst_sb, bias, tb):
            """dst_sb[:, ot, tb*512:...] = (W.T @ Xblk.T) + bias, bf16 out."""
            for ot in range(NOT):
                ps = ps_a.tile([P, TB], F32, tag="pj", name=f"p_{tb}_{ot}")
                for it in range(NIT):
                    nc.tensor.matmul(
                        ps[:],
                        w_sb[:, it, ot * P : (ot + 1) * P],
                        xt_blk[:, it],
                        start=(it == 0),
                        stop=(it == NIT - 1),
                    )
                nc.vector.tensor_scalar_add(
                    dst_sb[:, ot, tb * TB : (tb + 1) * TB],
                    ps[:],
                    bias[:, ot : ot + 1],
                )

        def k_proj_block(w_sb, xt_blk, tb):
            """Project the OWN K half's token block tb into kag_in{tb}."""
            for ot in range(NOT):
                ps = ps_a.tile([P, TB], F32, tag="pj", name=f"kp_{tb}_{ot}")
                for it in range(NIT):
                    nc.tensor.matmul(
                        ps[:],
                        w_sb[:, it, ot * P : (ot + 1) * P],
                        xt_blk[:, it],
                        start=(it == 0),
                        stop=(it == NIT - 1),
                    )
                st = cst_pool.tile([P, TB], BF16, tag="st16", name=f"ks_{tb}_{ot}")
                nc.vector.tensor_scalar_add(st[:], ps[:], bkt[:, ot : ot + 1])
                nc.sync.dma_start(out=io[f"kag_in{tb}"][:, ot], in_=st[:])

        def ag_k(tb):
            """AllGather the pair's K.T halves for own-block tb, land in kt_sb.

            Shard s of block tb covers global key tiles s*8+tb*4 .. +4.
            """
            nc.gpsimd.collective_compute(
                "AllGather",
                mybir.AluOpType.bypass,
                replica_groups=[[0, 1], [2, 3], [4, 5], [6, 7]],
                ins=[io[f"kag_in{tb}"].opt()],
                outs=[io[f"kag_out{tb}"].opt()],
            )
            for s in range(2):
                nc.sync.dma_start(
                    out=kt_sb[:, :, s * SQ + tb * TB : s * SQ + (tb + 1) * TB],
                    in_=io[f"kag_out{tb}"][s],
                )

        def v_proj_block(w_sb, xt_blk, tb):
            """Project the OWN V half's token block tb into vag_in{tb} (+bias)."""
            for ts in range(4):
                for ob in range(2):
                    ps = ps_a.tile([P, TB], F32, tag="pj", name=f"vp_{tb}_{ts}_{ob}")
                    for it in range(NIT):
                        nc.tensor.matmul(
                            ps[:],
                            xt_blk[:, it, ts * P : (ts + 1) * P],
                            w_sb[:, it, ob * TB : (ob + 1) * TB],
                            start=(it == 0),
                            stop=(it == NIT - 1),
                        )
                    st = cst_pool.tile(
                        [P, TB], BF16, tag="st16", name=f"vs_{tb}_{ts}_{ob}"
                    )
                    nc.vector.tensor_tensor(
                        st[:], ps[:], bv_bcast[:, ob * TB : (ob + 1) * TB], op=ADD
                    )
                    nc.sync.dma_start(
                        out=io[f"vag_in{tb}"][:, ts, ob * TB : (ob + 1) * TB],
                        in_=st[:],
                    )

        def ag_v(tb):
            """AllGather V halves for own-block tb, land strided-65 in v_sb."""
            nc.gpsimd.collective_compute(
                "AllGather",
                mybir.AluOpType.bypass,
                replica_groups=[[0, 1], [2, 3], [4, 5], [6, 7]],
                ins=[io[f"vag_in{tb}"].opt()],
                outs=[io[f"vag_out{tb}"].opt()],
            )
            for s in range(2):
                k0 = s * 8 + tb * 4
                dst = v_sb[:, k0 : k0 + 4, :].rearrange("p k (g c) -> p k g c", c=VW)
                nc.sync.dma_start(
                    out=dst[:, :, :, 0:HD],
                    in_=io[f"vag_out{tb}"][s].rearrange("p k (g c) -> p k g c", c=HD),
                )

        attn_state: dict = {}

        def emit_av(qb, pr, xs, pending):
            """AV matmuls for a (pexp, kt) unit produced by a prior exp."""
            pexp, kt = pending
            for h2 in range(2):
                nc.tensor.matmul(
                    xs[h2][:],
                    v_sb[:, kt, (2 * pr + h2) * VW : (2 * pr + h2 + 1) * VW],
                    pexp[:, h2 * TB : (h2 + 1) * TB],
                    start=(kt == 0),
                    stop=(kt == KT - 1),
                )

        def attn_chunk(qb, pr, kts):
            """Scores+exp+AV for key tiles `kts` of block (qb, pr).

            Each kt's AV is emitted during the NEXT kt's scores so the PE
            never FIFO-blocks on the exp.
            """
            if kts[0] == 0:
                attn_state[(qb, pr)] = (
                    [
                        x_psum.tile([VW, TB], F32, tag="xa", name=f"x_{qb}_{pr}_{h2}")
                        for h2 in range(2)
                    ],
                    [None],
                )
            xs, pend = attn_state[(qb, pr)]
            for kt in kts:
                sp = s_psum.tile([P, 1024], F32, tag="sp", name=f"sp_{qb}_{pr}_{kt}")
                for h2 in range(2):
                    nc.tensor.matmul(
                        sp[:, h2 * TB : (h2 + 1) * TB],
                        kt_sb[h2 * HD : (h2 + 1) * HD, pr, kt * P : (kt + 1) * P],
                        qt_sb[h2 * HD : (h2 + 1) * HD, pr, qb * TB : (qb + 1) * TB],
                        tile_position=(h2 * HD, 0),
                    )
                if pend[0] is not None:
                    emit_av(qb, pr, xs, pend[0])
                pexp = pe_pool.tile([P, 1024], BF16, tag="pe", name=f"pe_{qb}_{pr}_{kt}")
                nc.scalar.activation(pexp[:], sp[:], EXP, scale=1.0 / 8.0)
                pend[0] = (pexp, kt)
            if kts[-1] == KT - 1:
                emit_av(qb, pr, xs, pend[0])
                pend[0] = None
                xs = attn_state[(qb, pr)][0]
                for h2 in range(2):
                    rcp = rcp_pool.tile([1, TB], BF16, tag="rc", name=f"rc_{qb}_{pr}_{h2}")
                    with nc.allow_low_precision(
                        reason="bf16 softmax denominator reciprocal (0.4% el)"
                    ):
                        nc.vector.reciprocal(rcp[:], xs[h2][HD : HD + 1, :])
                    rb = rb_pool.tile([HD, TB], BF16, tag="rb", name=f"rb_{qb}_{pr}_{h2}")
                    nc.gpsimd.partition_broadcast(rb[:], rcp[0:1, :])
                    nc.vector.tensor_tensor(
                        xtn_sb[h2 * HD : (h2 + 1) * HD, pr, qb * TB : (qb + 1) * TB],
                        xs[h2][0:HD, :],
                        rb[:],
                        op=MULT,
                    )
                del attn_state[(qb, pr)]

        def c_chunk(qt, ob):
            """One [128, 512] tile of out = x.T.T @ Wo + bo."""
            ps = ps_a.tile([P, TB], F32, tag="pj", name=f"op_{qt}_{ob}")
            for pr in range(NP):
                nc.tensor.matmul(
                    ps[:],
                    xtn_sb[:, pr, qt * P : (qt + 1) * P],
                    wo_sb[:, pr, ob * TB : (ob + 1) * TB],
                    start=(pr == 0),
                    stop=(pr == NP - 1),
                )
            st = cst_pool.tile([P, TB], F32, tag="os", name=f"os_{qt}_{ob}")
            nc.vector.tensor_tensor(
                st[:], ps[:], bo_bcast[:, ob * TB : (ob + 1) * TB], op=ADD
            )
            nc.sync.dma_start(
                out=io["out"][qt * P : (qt + 1) * P, ob * TB : (ob + 1) * TB],
                in_=st[:],
            )

        # ---------------- pipelined emission ----------------
        # K halves project + AllGather first (the exchange is the long pole),
        # then Q(tb0) and the V halves; attention kt-chunks interleave in
        # gathered-arrival order (kt order is softmax-invariant).
        korder = [0, 1, 2, 3, 8, 9, 10, 11, 4, 5, 6, 7, 12, 13, 14, 15]

        wk = load_w("wk")
        xtk = xt_pool.tile([P, NIT, TB], BF16, tag="xt", name="xtk_0")
        transpose_block(io["xk"], 0, xtk)
        k_proj_block(wk, xtk, 0)
        ag_k(0)
        xtk = xt_pool.tile([P, NIT, TB], BF16, tag="xt", name="xtk_1")
        transpose_block(io["xk"], TB, xtk)
        k_proj_block(wk, xtk, 1)
        ag_k(1)

        wq = load_w("wq")
        xtq0 = xt_pool.tile([P, NIT, TB], BF16, tag="xt", name="xtq_0")
        transpose_block(io["xq"], 0, xtq0)
        qk_proj_block(wq, xtq0, qt_sb, bqt, 0)

        wv = load_w("wv")
        xtv = xt_pool.tile([P, NIT, TB], BF16, tag="xt", name="xtv_0")
        transpose_block(io["xv"], 0, xtv)
        v_proj_block(wv, xtv, 0)
        ag_v(0)

        attn_chunk(0, 0, korder[0:4])
        xtv = xt_pool.tile([P, NIT, TB], BF16, tag="xt", name="xtv_1")
        transpose_block(io["xv"], TB, xtv)
        v_proj_block(wv, xtv, 1)
        ag_v(1)
        attn_chunk(0, 0, korder[4:8])
        attn_chunk(0, 0, korder[8:12])
        attn_chunk(0, 0, korder[12:16])

        # remaining attention, with the rest of phase A and the qb0 output
        # projection spread through the B blocks as PE filler
        attn_chunk(0, 1, korder)
        xtq1 = xt_pool.tile([P, NIT, TB], BF16, tag="xt", name="xtq_1")
        transpose_block(io["xq"], TB, xtq1)
        attn_chunk(0, 2, korder)
        qk_proj_block(wq, xtq1, qt_sb, bqt, 1)
        attn_chunk(0, 3, korder)
        for it in range(NP):
            nc.sync.dma_start(
                out=wo_sb[:, it], in_=io["wo"][it * P : (it + 1) * P, :]
            )
        for pr in range(4, NP):
            attn_chunk(0, pr, korder)
        for pr in range(NP):
            attn_chunk(1, pr, korder)
            if pr == 1:
                c_chunk(0, 0)
                c_chunk(0, 1)
            elif pr >= 2 and pr <= 4:
                c_chunk(pr - 1, 0)
                c_chunk(pr - 1, 1)
        for qt in range(4, 8):
            c_chunk(qt, 0)
            c_chunk(qt, 1)


def build_module(reps=1):
    key = ("nc", reps)
    if key in _CACHE:
        return _CACHE[key]
    nc = bacc.Bacc("TRN2", target_bir_lowering=False, debug=False, num_devices=8)
    io = {}
    io["xq"] = nc.dram_tensor("xq", [SQ, D], BF16, kind="ExternalInput").ap()
    io["xk"] = nc.dram_tensor("xk", [SQ, D], BF16, kind="ExternalInput").ap()
    io["xv"] = nc.dram_tensor("xv", [SQ, D], BF16, kind="ExternalInput").ap()
    for w in ("wq", "wk", "wv", "wo"):
        io[w] = nc.dram_tensor(w, [D, D], BF16, kind="ExternalInput").ap()
    for b in ("bq", "bk"):
        io[b] = nc.dram_tensor(b, [D], F32, kind="ExternalInput").ap()
    for b in ("bv", "bo"):
        io[b] = nc.dram_tensor(b, [D], BF16, kind="ExternalInput").ap()
    io["out"] = nc.dram_tensor("out", [SQ, D], F32, kind="ExternalOutput").ap()

    with tile.TileContext(nc) as tc:
        for rep in range(reps):
            t = f"_r{rep}" if rep else ""
            for tb in range(2):
                io[f"kag_in{tb}"] = nc.dram_tensor(
                    f"kag_in{tb}{t}", [P, NP, TB], BF16
                ).ap()
                io[f"kag_out{tb}"] = nc.dram_tensor(
                    f"kag_out{tb}{t}", [2, P, NP, TB], BF16
                ).ap()
                io[f"vag_in{tb}"] = nc.dram_tensor(
                    f"vag_in{tb}{t}", [P, 4, D], BF16
                ).ap()
                io[f"vag_out{tb}"] = nc.dram_tensor(
                    f"vag_out{tb}{t}", [2, P, 4, D], BF16
                ).ap()
            _emit(tc, io, tag=t)
    nc.compile()
    _CACHE[key] = nc
    return nc


def _bf16(x):
    import ml_dtypes

    return np.ascontiguousarray(np.asarray(x, np.float32).astype(ml_dtypes.bfloat16))


def make_in_maps(query, key, value, Wq, bq, Wk, bk, Wv, bv, Wo, bo):
    shared = {
        "wq": _bf16(Wq),
        "wk": _bf16(Wk),
        "wv": _bf16(Wv),
        "wo": _bf16(Wo),
        "bq": np.ascontiguousarray(np.asarray(bq, np.float32)),
        "bk": np.ascontiguousarray(np.asarray(bk, np.float32)),
        "bv": _bf16(bv),
        "bo": _bf16(bo),
    }
    qb, kb, vb = _bf16(query), _bf16(key), _bf16(value)
    in_maps = []
    for c in range(8):
        b, qh = divmod(c, 2)
        in_maps.append(
            {
                "xq": np.ascontiguousarray(qb[b, qh * SQ : (qh + 1) * SQ]),
                "xk": np.ascontiguousarray(kb[b, qh * SQ : (qh + 1) * SQ]),
                "xv": np.ascontiguousarray(vb[b, qh * SQ : (qh + 1) * SQ]),
                **shared,
            }
        )
    return in_maps


LAST_RESULTS = None


def kernel(query, key, value, Wq, bq, Wk, bk, Wv, bv, Wo, bo):
    global LAST_RESULTS
    nc = build_module()
    in_maps = make_in_maps(query, key, value, Wq, bq, Wk, bk, Wv, bv, Wo, bo)
    try:
        res = run_bass_kernel_spmd(nc, in_maps, core_ids=list(range(8)))
    except ModuleNotFoundError:
        # BASS_TRACE was requested but this container lacks the axon NTFF
        # profiling hook module; rerun with tracing disabled.
        os.environ["BASS_NEVER_TRACE"] = "1"
        res = run_bass_kernel_spmd(nc, in_maps, core_ids=list(range(8)))
    LAST_RESULTS = res
    out = np.empty((B, S, D), np.float32)
    for c in range(8):
        b, qh = divmod(c, 2)
        out[b, qh * SQ : (qh + 1) * SQ] = res.results[c]["out"]
    return out
